# revision 15
# baseline (speedup 1.0000x reference)
"""GAT model (2x GATConv 8 heads x 32 + mean-pool + MLP) as a Bass/Tile kernel
on 8 trn2 NeuronCores.

Distribution: both GAT layers are dst-sharded (core k owns dst nodes
[8192k, 8192(k+1)), 64 blocks of 128). Per-edge data moves via dma_gather on
4 rotated SWDGE queues. Softmax uses the identity
  exp(leaky_relu(el_s + er_d)) = q_d * max(p_s, u_s * t_d),
  p = exp(el), u = exp(0.2 el), t = exp(-0.8 er),
where the q_d factor cancels between numerator and denominator, so only
src-side (p, u — gathered with h in one 768B table row) and one dst-side
vector t (gathered from a per-core 256B-row table) are needed. Max-subtraction
cancels algebraically and is skipped. Accumulation is a one-hot matmul into
PSUM [denom | sum m~ h]. The layer boundary is one 4.4MB/rank AllGather;
pooling accumulates pooled^T partials in PSUM via global-graph-slot one-hot
matmuls, combined with a 512KB AllReduce; the MLP runs redundantly per core.
"""

import numpy as np
import ml_dtypes

N_CORES = 8
N = 65536
E = 1048576
G = 512
F_IN = 128
H = 8
D = 32
F_MID = 256
P_HID = 512

NODES_PER_CORE = N // N_CORES  # 8192
BLOCKS = NODES_PER_CORE // 128  # 64
HALF = N // 2  # 32768

ROW_W = 384  # table row: [h(256) | p(8) | u(8) | pad] bf16 = 768B
T_W = 128  # t-table row: [t(8) | pad] bf16 = 256B

BF16 = ml_dtypes.bfloat16


def _wrap16(a):
    """[n] int -> [128, n/16] int16, 16-wrapped (i at [i%16, i//16]), rows
    replicated over the 8 16-partition groups."""
    n = a.shape[0]
    w = a.reshape(n // 16, 16).T.astype(np.int16)
    return np.tile(w, (8, 1))


def _prep_core(k, src, dst, graph_ids, hc):
    """Host-side per-core edge grids. Returns dict of numpy arrays."""
    lo_k = NODES_PER_CORE * k
    sel = np.nonzero((dst >= lo_k) & (dst < lo_k + NODES_PER_CORE))[0]
    s, d = src[sel], dst[sel]
    order = np.argsort(d, kind="stable")
    s, d = s[order], d[order]
    dloc = d - lo_k

    idx_lo = np.zeros((BLOCKS, 128 * hc), dtype=np.int64)
    idx_hi = np.zeros((BLOCKS, 128 * hc), dtype=np.int64)
    idx_t = np.zeros((BLOCKS, 2 * 128 * hc), dtype=np.int64)
    dstloc = np.full((128, BLOCKS * 2 * hc), 300.0, dtype=np.float32)

    blk = dloc // 128
    for b in range(BLOCKS):
        m = blk == b
        sb, db = s[m], dloc[m]
        is_lo = sb < HALF
        slo, dlo_ = sb[is_lo], db[is_lo]
        shi, dhi_ = sb[~is_lo] - HALF, db[~is_lo]
        nlo, nhi = len(slo), len(shi)
        assert nlo <= 128 * hc and nhi <= 128 * hc, (nlo, nhi)
        gl = np.zeros(128 * hc, dtype=np.int64)
        gl[:nlo] = slo
        gh = np.zeros(128 * hc, dtype=np.int64)
        gh[:nhi] = shi
        idx_lo[b] = gl
        idx_hi[b] = gh
        # t-gather idx: local dst row; order = [lo-half slots | hi-half slots]
        gt = np.zeros(2 * 128 * hc, dtype=np.int64)
        gt[:nlo] = dlo_
        gt[128 * hc : 128 * hc + nhi] = dhi_
        idx_t[b] = gt
        # dst-local slot values for S build (pad = 300 -> zero one-hot col)
        dl = np.full(2 * 128 * hc, 300, dtype=np.int64)
        dl[:nlo] = dlo_ % 128
        dl[128 * hc : 128 * hc + nhi] = dhi_ % 128
        dstloc[:, b * 2 * hc : (b + 1) * 2 * hc] = (
            dl.reshape(2 * hc, 128).T.astype(np.float32)
        )

    gnode = graph_ids[lo_k : lo_k + NODES_PER_CORE].astype(np.float32)
    return {
        "idx_lo": np.concatenate([_wrap16(idx_lo[b]) for b in range(BLOCKS)], 1),
        "idx_hi": np.concatenate([_wrap16(idx_hi[b]) for b in range(BLOCKS)], 1),
        "idx_t": np.concatenate([_wrap16(idx_t[b]) for b in range(BLOCKS)], 1),
        "dstloc": dstloc.astype(BF16),
        "gnode": gnode.reshape(BLOCKS, 128).T.astype(np.float32).copy(),  # [128, BLOCKS]
    }


def _build(hc):
    import concourse.bass as bass
    import concourse.bacc as bacc
    import concourse.mybir as mybir
    import concourse.tile as tile
    from concourse.masks import make_identity

    fp32 = mybir.dt.float32
    bf16 = mybir.dt.bfloat16
    i16 = mybir.dt.int16
    AF = mybir.ActivationFunctionType

    C2 = 2 * hc  # cols per block
    W16 = 128 * hc // 16  # idx cols per half-gather
    TW16 = 2 * W16

    nc = bacc.Bacc(
        "TRN2",
        target_bir_lowering=False,
        debug=False,
        num_devices=N_CORES,
        num_swdge_queues=4,
    )

    # ---- inputs ----
    xb = nc.dram_tensor("xb", [N, F_IN], bf16, kind="ExternalInput")
    xown = nc.dram_tensor("xown", [NODES_PER_CORE, F_IN], bf16, kind="ExternalInput")
    r0 = nc.dram_tensor("r0", [F_IN, 272], bf16, kind="ExternalInput")
    r1 = nc.dram_tensor("r1", [F_MID, 272], bf16, kind="ExternalInput")
    c0r = nc.dram_tensor("c0r", [F_IN, 8], bf16, kind="ExternalInput")
    wd1 = nc.dram_tensor("wd1", [F_MID, P_HID], bf16, kind="ExternalInput")
    bd1 = nc.dram_tensor("bd1", [P_HID, 1], fp32, kind="ExternalInput")
    wd2 = nc.dram_tensor("wd2", [P_HID, 1], bf16, kind="ExternalInput")
    bd2 = nc.dram_tensor("bd2", [1, 1], fp32, kind="ExternalInput")
    cntr = nc.dram_tensor("cntr", [128, G], fp32, kind="ExternalInput")
    iota128 = nc.dram_tensor("iota128", [128, 128], bf16, kind="ExternalInput")
    iota512 = nc.dram_tensor("iota512", [128, G], fp32, kind="ExternalInput")
    d_idx_lo = nc.dram_tensor("idx_lo", [128, BLOCKS * W16], i16, kind="ExternalInput")
    d_idx_hi = nc.dram_tensor("idx_hi", [128, BLOCKS * W16], i16, kind="ExternalInput")
    d_idx_t = nc.dram_tensor("idx_t", [128, BLOCKS * TW16], i16, kind="ExternalInput")
    d_dstloc = nc.dram_tensor("dstloc", [128, BLOCKS * C2], bf16, kind="ExternalInput")
    d_gnode = nc.dram_tensor("gnode", [128, BLOCKS], fp32, kind="ExternalInput")

    out_d = nc.dram_tensor("out", [G, 1], fp32, kind="ExternalOutput")
    import os
    _dbg = bool(os.environ.get("GAT_DEBUG"))
    if _dbg:
        dbg_tab0 = nc.dram_tensor("dbg_tab0", [512, ROW_W], bf16, kind="ExternalOutput")
        dbg_t0 = nc.dram_tensor("dbg_t0", [512, T_W], bf16, kind="ExternalOutput")
        dbg_slab = nc.dram_tensor("dbg_slab", [NODES_PER_CORE, ROW_W], bf16, kind="ExternalOutput")
        dbg_pool = nc.dram_tensor("dbg_pool", [F_MID, G], fp32, kind="ExternalOutput")

    # ---- internal DRAM ----
    t0tab = nc.dram_tensor("t0tab", [NODES_PER_CORE, T_W], bf16, kind="Internal")
    t1tab = nc.dram_tensor("t1tab", [NODES_PER_CORE, T_W], bf16, kind="Internal")
    tab0 = nc.dram_tensor("tab0", [N, ROW_W], bf16, kind="Internal")

    with tile.TileContext(nc) as tc:
        with (
            tc.tile_pool(name="const", bufs=1) as cp,
            tc.tile_pool(name="idxp", bufs=1) as ip,
            tc.tile_pool(name="xt", bufs=4) as xtp,
            tc.tile_pool(name="stage", bufs=4) as stp,
            tc.tile_pool(name="gath", bufs=4) as gp,
            tc.tile_pool(name="work", bufs=3) as wp,
            tc.tile_pool(name="mlp", bufs=1) as mp,
            tc.tile_pool(name="ps", bufs=1, space="PSUM") as pp,
            tc.tile_pool(name="psacc", bufs=1, space="PSUM") as pacc,
            tc.tile_pool(name="dram", bufs=1, space="DRAM") as dp,
        ):
            # ---- resident constants ----
            r0t = cp.tile([F_IN, 272], bf16)
            nc.sync.dma_start(out=r0t[:], in_=r0[:])
            r1t = cp.tile([128, 2, 272], bf16)
            nc.sync.dma_start(
                out=r1t[:], in_=r1[:].rearrange("(h p) w -> p h w", p=128)
            )
            c0rt = cp.tile([F_IN, 8], bf16)
            nc.sync.dma_start(out=c0rt[:], in_=c0r[:])
            io128 = cp.tile([128, 128], bf16)
            nc.sync.dma_start(out=io128[:], in_=iota128[:])
            io512 = cp.tile([128, G], fp32)
            nc.sync.dma_start(out=io512[:], in_=iota512[:])
            ident = cp.tile([128, 128], bf16)
            make_identity(nc, ident[:])
            ilo = ip.tile([128, BLOCKS * W16], i16)
            nc.sync.dma_start(out=ilo[:], in_=d_idx_lo[:])
            ihi = ip.tile([128, BLOCKS * W16], i16)
            nc.sync.dma_start(out=ihi[:], in_=d_idx_hi[:])
            it = ip.tile([128, BLOCKS * TW16], i16)
            nc.sync.dma_start(out=it[:], in_=d_idx_t[:])
            dlo = ip.tile([128, BLOCKS * C2], bf16)
            nc.sync.dma_start(out=dlo[:], in_=d_dstloc[:])
            gnd = ip.tile([128, BLOCKS], fp32)
            nc.sync.dma_start(out=gnd[:], in_=d_gnode[:])

            # ---- stage A: T0 build (all 65536 rows, redundant per core) ----
            # X^T tiles of 512 nodes; per tile 4 matmuls vs r0 -> [h|el|er]
            xview = xb[:].rearrange("(a n) f -> a n f", n=512)
            t0view = tab0[:].rearrange("(a g p) w -> a g p w", g=4, p=128)
            for a in range(N // 512):
                xt = xtp.tile([F_IN, 512], bf16, tag="xt")
                nc.sync.dma_start(out=xt[:], in_=xview[a], transpose=True)
                st = stp.tile([128, 4, ROW_W], bf16, tag="t0st")
                for g in range(4):
                    ps = pp.tile([128, 272], fp32, tag="mmbig", bufs=4)
                    nc.tensor.matmul(
                        out=ps[:],
                        lhsT=xt[:, g * 128 : (g + 1) * 128],
                        rhs=r0t[:],
                        start=True,
                        stop=True,
                    )
                    nc.vector.tensor_copy(out=st[:, g, 0:256], in_=ps[:, 0:256])
                    nc.scalar.activation(
                        out=st[:, g, 256:264], in_=ps[:, 256:264], func=AF.Exp
                    )
                    nc.scalar.activation(
                        out=st[:, g, 264:272], in_=ps[:, 256:264], func=AF.Exp,
                        scale=0.2,
                    )
                nc.sync.dma_start(out=t0view[a].rearrange("g p w -> p g w"), in_=st[:])

            # t0 table for own dst range, from xown
            xoview = xown[:].rearrange("(a n) f -> a n f", n=512)
            t0tview = t0tab[:].rearrange("(a g p) w -> a g p w", g=4, p=128)
            for a in range(NODES_PER_CORE // 512):
                xt = xtp.tile([F_IN, 512], bf16, tag="xt")
                nc.sync.dma_start(out=xt[:], in_=xoview[a], transpose=True)
                st = stp.tile([128, 4, T_W], bf16, tag="tst")
                for g in range(4):
                    ps = pp.tile([128, 8], fp32, tag="small", bufs=2)
                    nc.tensor.matmul(
                        out=ps[:],
                        lhsT=xt[:, g * 128 : (g + 1) * 128],
                        rhs=c0rt[:],
                        start=True,
                        stop=True,
                    )
                    nc.scalar.activation(
                        out=st[:, g, 0:8], in_=ps[:], func=AF.Exp, scale=-0.8
                    )
                nc.sync.dma_start(out=t0tview[a].rearrange("g p w -> p g w"), in_=st[:])

            # ---- layer loops ----
            t1slab = dp.tile([NODES_PER_CORE, ROW_W], bf16)
            t1full = dp.tile([N, ROW_W], bf16)
            poolin = dp.tile([F_MID, G], fp32)
            poolout = dp.tile([F_MID, G], fp32)

            def gat_layer(tab_lo, tab_hi, ttab, layer):
                """One GAT layer over 64 dst blocks."""
                pooled = None
                if layer == 1:
                    pool_a = pacc.tile([128, G], fp32, tag="pool_a")
                    pool_b = pacc.tile([128, G], fp32, tag="pool_b")
                    pooled = [pool_a, pool_b]
                t1view = t1slab[:].rearrange("(b p) w -> b p w", p=128)
                t1tview = t1tab[:].rearrange("(b p) w -> b p w", p=128)
                for b in range(BLOCKS):
                    g = gp.tile([128, C2, ROW_W], bf16, tag="g")
                    nc.gpsimd.dma_gather(
                        out_ap=g[:, :hc, :],
                        in_ap=tab_lo,
                        idxs_ap=ilo[:, b * W16 : (b + 1) * W16],
                        num_idxs=128 * hc,
                        num_idxs_reg=128 * hc,
                        elem_size=ROW_W,
                        single_packet=False,
                        queue_num=(3 * b) % 4,
                    )
                    nc.gpsimd.dma_gather(
                        out_ap=g[:, hc:, :],
                        in_ap=tab_hi,
                        idxs_ap=ihi[:, b * W16 : (b + 1) * W16],
                        num_idxs=128 * hc,
                        num_idxs_reg=128 * hc,
                        elem_size=ROW_W,
                        single_packet=False,
                        queue_num=(3 * b + 1) % 4,
                    )
                    tb = gp.tile([128, C2, T_W], bf16, tag="tb")
                    nc.gpsimd.dma_gather(
                        out_ap=tb[:],
                        in_ap=ttab,
                        idxs_ap=it[:, b * TW16 : (b + 1) * TW16],
                        num_idxs=128 * C2,
                        num_idxs_reg=128 * C2,
                        elem_size=T_W,
                        single_packet=False,
                        queue_num=(3 * b + 2) % 4,
                    )
                    # m~ = max(p, u*t) -> rhs[:, :, 0:8]
                    rhs = wp.tile([128, C2, 264], bf16, tag="rhs")
                    ut = wp.tile([128, C2, 8], fp32, tag="ut")
                    nc.vector.tensor_tensor(
                        out=ut[:],
                        in0=g[:, :, 264:272],
                        in1=tb[:, :, 0:8],
                        op=mybir.AluOpType.mult,
                    )
                    nc.vector.tensor_tensor(
                        out=rhs[:, :, 0:8],
                        in0=ut[:],
                        in1=g[:, :, 256:264],
                        op=mybir.AluOpType.max,
                    )
                    # msg = m~ (bcast over 32) * h -> rhs[:, :, 8:264]
                    mb_ = rhs[:, :, 0:8][:, :, :, None].to_broadcast([128, C2, 8, 32])
                    nc.vector.tensor_tensor(
                        out=rhs[:, :, 8:264].rearrange("p c (h d) -> p c h d", d=32),
                        in0=g[:, :, 0:256].rearrange("p c (h d) -> p c h d", d=32),
                        in1=mb_,
                        op=mybir.AluOpType.mult,
                    )
                    # S one-hot [128, C2, 128]
                    s = wp.tile([128, C2, 128], bf16, tag="s")
                    nc.vector.tensor_tensor(
                        out=s[:],
                        in0=dlo[:, b * C2 : (b + 1) * C2][:, :, None]
                        .to_broadcast([128, C2, 128]),
                        in1=io128[:][:, None, :].to_broadcast([128, C2, 128]),
                        op=mybir.AluOpType.is_equal,
                    )
                    acc = pp.tile([128, 264], fp32, tag="mmbig", bufs=4)
                    for c in range(C2):
                        nc.tensor.matmul(
                            out=acc[:],
                            lhsT=s[:, c, :],
                            rhs=rhs[:, c, :],
                            start=(c == 0),
                            stop=(c == C2 - 1),
                        )
                    # normalize: h_out = acc[:, 8:264] / denom
                    rec = wp.tile([128, 8], fp32, tag="rec")
                    nc.vector.tensor_scalar_max(out=rec[:], in0=acc[:, 0:8], scalar1=1e-30)
                    nc.vector.reciprocal(out=rec[:], in_=rec[:])
                    hout = wp.tile([128, F_MID], bf16, tag="hout")
                    nc.vector.tensor_tensor(
                        out=hout[:].rearrange("p (h d) -> p h d", d=32),
                        in0=acc[:, 8:264].rearrange("p (h d) -> p h d", d=32),
                        in1=rec[:][:, :, None].to_broadcast([128, 8, 32]),
                        op=mybir.AluOpType.mult,
                    )
                    if layer == 0:
                        # h^T via PE transpose, then [h1|el1|er1] = h^T.T @ r1
                        hT = wp.tile([128, 2, 128], bf16, tag="hT")
                        for fh in range(2):
                            pst = pp.tile([128, 128], bf16, tag="small", bufs=2)
                            nc.tensor.transpose(
                                out=pst[:],
                                in_=hout[:, fh * 128 : (fh + 1) * 128],
                                identity=ident[:],
                            )
                            nc.vector.tensor_copy(out=hT[:, fh, :], in_=pst[:])
                        ps2 = pp.tile([128, 272], fp32, tag="mmbig", bufs=4)
                        for fh in range(2):
                            nc.tensor.matmul(
                                out=ps2[:],
                                lhsT=hT[:, fh, :],
                                rhs=r1t[:, fh, :],
                                start=(fh == 0),
                                stop=(fh == 1),
                            )
                        st = stp.tile([128, 272], bf16, tag="t1st")
                        nc.vector.tensor_copy(out=st[:, 0:256], in_=ps2[:, 0:256])
                        nc.scalar.activation(
                            out=st[:, 256:264], in_=ps2[:, 256:264], func=AF.Exp
                        )
                        nc.scalar.activation(
                            out=st[:, 264:272],
                            in_=ps2[:, 256:264],
                            func=AF.Exp,
                            scale=0.2,
                        )
                        tst = stp.tile([128, 8], bf16, tag="tt1")
                        nc.scalar.activation(
                            out=tst[:], in_=ps2[:, 264:272], func=AF.Exp, scale=-0.8
                        )
                        nc.sync.dma_start(out=t1view[b, :, 0:272], in_=st[:])
                        nc.sync.dma_start(out=t1tview[b, :, 0:8], in_=tst[:])
                    else:
                        # pooling: pooledT[f, g] += hout^T-slices @ P
                        pmat = wp.tile([128, G], bf16, tag="pmat")
                        nc.vector.tensor_tensor(
                            out=pmat[:],
                            in0=gnd[:, b : b + 1].to_broadcast([128, G]),
                            in1=io512[:],
                            op=mybir.AluOpType.is_equal,
                        )
                        for fh in range(2):
                            nc.tensor.matmul(
                                out=pooled[fh][:],
                                lhsT=hout[:, fh * 128 : (fh + 1) * 128],
                                rhs=pmat[:],
                                start=(b == 0),
                                stop=(b == BLOCKS - 1),
                            )
                return pooled

            gat_layer(tab0[:HALF, :], tab0[HALF:, :], t0tab[:], 0)

            # AllGather slab -> full table
            nc.gpsimd.collective_compute(
                "AllGather",
                mybir.AluOpType.bypass,
                ins=[t1slab.opt()],
                outs=[t1full.opt()],
                replica_groups=[list(range(N_CORES))],
            )

            pooled = gat_layer(
                t1full[:][:HALF, :], t1full[:][HALF:, :], t1tab[:], 1
            )

            if _dbg:
                nc.sync.dma_start(out=dbg_tab0[:], in_=tab0[0:512, :])
                nc.sync.dma_start(out=dbg_t0[:], in_=t0tab[0:512, :])
                nc.sync.dma_start(out=dbg_slab[:], in_=t1slab[:])

            # ---- pooled AllReduce ----
            pst = mp.tile([128, 2, G], fp32)
            nc.vector.tensor_copy(out=pst[:, 0, :], in_=pooled[0][:])
            nc.vector.tensor_copy(out=pst[:, 1, :], in_=pooled[1][:])
            nc.sync.dma_start(
                out=poolin[:].rearrange("(h p) g -> p h g", p=128), in_=pst[:]
            )
            nc.gpsimd.collective_compute(
                "AllReduce",
                mybir.AluOpType.add,
                ins=[poolin.opt()],
                outs=[poolout.opt()],
                replica_groups=[list(range(N_CORES))],
            )
            if _dbg:
                nc.sync.dma_start(out=dbg_pool[:], in_=poolin[:])

            # ---- MLP ----
            cnt = mp.tile([128, G], fp32)
            nc.sync.dma_start(out=cnt[:], in_=cntr[:])
            mean = mp.tile([128, 2, G], bf16)
            pr = mp.tile([128, 2, G], fp32)
            nc.sync.dma_start(
                out=pr[:], in_=poolout[:].rearrange("(h p) g -> p h g", p=128)
            )
            for fh in range(2):
                nc.vector.tensor_tensor(
                    out=mean[:, fh, :],
                    in0=pr[:, fh, :],
                    in1=cnt[:],
                    op=mybir.AluOpType.mult,
                )
            w1t = mp.tile([128, 2, P_HID], bf16)
            nc.sync.dma_start(
                out=w1t[:], in_=wd1[:].rearrange("(h p) j -> p h j", p=128)
            )
            b1t = mp.tile([128, 4, 1], fp32)
            nc.sync.dma_start(
                out=b1t[:], in_=bd1[:].rearrange("(q p) o -> p q o", p=128)
            )
            w2t = mp.tile([128, 4, 1], bf16)
            nc.sync.dma_start(
                out=w2t[:], in_=wd2[:].rearrange("(q p) o -> p q o", p=128)
            )
            b2t = mp.tile([1, 1], fp32)
            nc.sync.dma_start(out=b2t[:], in_=bd2[:])
            hid = mp.tile([128, 4, G], bf16)
            for q in range(4):
                psh = pp.tile([128, G], fp32, tag="mmbig", bufs=4)
                for fh in range(2):
                    nc.tensor.matmul(
                        out=psh[:],
                        lhsT=w1t[:, fh, q * 128 : (q + 1) * 128],
                        rhs=mean[:, fh, :],
                        start=(fh == 0),
                        stop=(fh == 1),
                    )
                nc.scalar.activation(
                    out=hid[:, q, :],
                    in_=psh[:],
                    func=AF.Relu,
                    bias=b1t[:, q, :],
                )
            pso = pp.tile([1, G], fp32, tag="small", bufs=2)
            for q in range(4):
                nc.tensor.matmul(
                    out=pso[:],
                    lhsT=w2t[:, q, :],
                    rhs=hid[:, q, :],
                    start=(q == 0),
                    stop=(q == 3),
                )
            ot = mp.tile([1, G], fp32)
            nc.vector.tensor_scalar_add(out=ot[:], in0=pso[:], scalar1=b2t[:])
            nc.sync.dma_start(out=out_d[:].rearrange("g o -> o g"), in_=ot[:])

    nc.compile()
    return nc


_CACHED = {}


def kernel(node_feats, src, dst, graph_ids, num_graphs,
           W0, al0, ar0, W1, al1, ar1, Wd1, bd1, Wd2, bd2,
           _trace=False):
    from concourse.bass_utils import run_bass_kernel_spmd

    src = np.asarray(src).astype(np.int64)
    dst = np.asarray(dst).astype(np.int64)
    graph_ids = np.asarray(graph_ids).astype(np.int64)
    node_feats = np.asarray(node_feats, dtype=np.float32)

    # ---- host prep ----
    # attention-projection matrices folded into the fc weights
    def _amat(al):
        a = np.zeros((F_MID, H), np.float32)
        for h_ in range(H):
            a[h_ * D : (h_ + 1) * D, h_] = al[h_]
        return a

    W0 = np.asarray(W0, np.float32)
    W1 = np.asarray(W1, np.float32)
    r0 = np.concatenate(
        [W0, W0 @ _amat(np.asarray(al0, np.float32)),
         W0 @ _amat(np.asarray(ar0, np.float32))], axis=1
    ).astype(BF16)
    r1 = np.concatenate(
        [W1, W1 @ _amat(np.asarray(al1, np.float32)),
         W1 @ _amat(np.asarray(ar1, np.float32))], axis=1
    ).astype(BF16)
    c0r = (W0 @ _amat(np.asarray(ar0, np.float32))).astype(BF16)

    cnt = np.bincount(graph_ids, minlength=G).astype(np.float32)
    cntr = np.tile((1.0 / np.maximum(cnt, 1.0))[None, :], (128, 1)).astype(np.float32)
    iota128 = np.tile(np.arange(128, dtype=np.float32)[None, :], (128, 1)).astype(BF16)
    iota512 = np.tile(np.arange(G, dtype=np.float32)[None, :], (128, 1))
    xb = node_feats.astype(BF16)

    # per-(core, block, half) max edge count -> hc
    counts = np.zeros((N_CORES, BLOCKS, 2), np.int64)
    core = dst // NODES_PER_CORE
    blk = (dst % NODES_PER_CORE) // 128
    half = (src >= HALF).astype(np.int64)
    np.add.at(counts, (core, blk, half), 1)
    hc = int(np.ceil(counts.max() / 128))

    key = hc
    if key not in _CACHED:
        _CACHED[key] = _build(hc)
    nc = _CACHED[key]

    shared = {
        "xb": xb, "r0": r0, "r1": r1, "c0r": c0r,
        "wd1": np.asarray(Wd1, np.float32).astype(BF16),
        "bd1": np.asarray(bd1, np.float32).reshape(P_HID, 1),
        "wd2": np.asarray(Wd2, np.float32).astype(BF16).reshape(P_HID, 1),
        "bd2": np.asarray(bd2, np.float32).reshape(1, 1),
        "cntr": cntr, "iota128": iota128, "iota512": iota512,
    }
    in_maps = []
    for k in range(N_CORES):
        m = dict(shared)
        m["xown"] = xb[k * NODES_PER_CORE : (k + 1) * NODES_PER_CORE]
        m.update(_prep_core(k, src, dst, graph_ids, hc))
        in_maps.append(m)

    res = run_bass_kernel_spmd(
        nc, in_maps, core_ids=list(range(N_CORES)), trace=_trace
    )
    out = res.results[0]["out"].astype(np.float32)
    if _trace:
        kernel._last_exec_ns = res.exec_time_ns
    return out


# revision 16
# speedup vs baseline: 1.0148x; 1.0148x over previous
"""GAT model (2x GATConv 8 heads x 32 + mean-pool + MLP) as a Bass/Tile kernel
on 8 trn2 NeuronCores.

Distribution: both GAT layers are dst-sharded (core k owns dst nodes
[8192k, 8192(k+1)), 64 blocks of 128). Per-edge data moves via dma_gather on
4 rotated SWDGE queues. Softmax uses the identity
  exp(leaky_relu(el_s + er_d)) = q_d * max(p_s, u_s * t_d),
  p = exp(el), u = exp(0.2 el), t = exp(-0.8 er),
where the q_d factor cancels between numerator and denominator, so only
src-side (p, u — gathered with h in one 768B table row) and one dst-side
vector t (gathered from a per-core 256B-row table) are needed. Max-subtraction
cancels algebraically and is skipped. Accumulation is a one-hot matmul into
PSUM [denom | sum m~ h]. The layer boundary is one 4.4MB/rank AllGather;
pooling accumulates pooled^T partials in PSUM via global-graph-slot one-hot
matmuls, combined with a 512KB AllReduce; the MLP runs redundantly per core.
"""

import numpy as np
import ml_dtypes

N_CORES = 8
N = 65536
E = 1048576
G = 512
F_IN = 128
H = 8
D = 32
F_MID = 256
P_HID = 512

NODES_PER_CORE = N // N_CORES  # 8192
BLOCKS = NODES_PER_CORE // 128  # 64
HALF = N // 2  # 32768

ROW_W = 384  # table row: [h(256) | p(8) | u(8) | pad] bf16 = 768B
T_W = 128  # t-table row: [t(8) | pad] bf16 = 256B

BF16 = ml_dtypes.bfloat16


def _wrap16(a):
    """[n] int -> [128, n/16] int16, 16-wrapped (i at [i%16, i//16]), rows
    replicated over the 8 16-partition groups."""
    n = a.shape[0]
    w = a.reshape(n // 16, 16).T.astype(np.int16)
    return np.tile(w, (8, 1))


def _prep_core(k, src, dst, graph_ids, hc):
    """Host-side per-core edge grids. Returns dict of numpy arrays."""
    lo_k = NODES_PER_CORE * k
    sel = np.nonzero((dst >= lo_k) & (dst < lo_k + NODES_PER_CORE))[0]
    s, d = src[sel], dst[sel]
    order = np.argsort(d, kind="stable")
    s, d = s[order], d[order]
    dloc = d - lo_k

    idx_lo = np.zeros((BLOCKS, 128 * hc), dtype=np.int64)
    idx_hi = np.zeros((BLOCKS, 128 * hc), dtype=np.int64)
    idx_t = np.zeros((BLOCKS, 2 * 128 * hc), dtype=np.int64)
    dstloc = np.full((128, BLOCKS * 2 * hc), 300.0, dtype=np.float32)

    blk = dloc // 128
    for b in range(BLOCKS):
        m = blk == b
        sb, db = s[m], dloc[m]
        is_lo = sb < HALF
        slo, dlo_ = sb[is_lo], db[is_lo]
        shi, dhi_ = sb[~is_lo] - HALF, db[~is_lo]
        nlo, nhi = len(slo), len(shi)
        assert nlo <= 128 * hc and nhi <= 128 * hc, (nlo, nhi)
        gl = np.zeros(128 * hc, dtype=np.int64)
        gl[:nlo] = slo
        gh = np.zeros(128 * hc, dtype=np.int64)
        gh[:nhi] = shi
        idx_lo[b] = gl
        idx_hi[b] = gh
        # t-gather idx: local dst row; order = [lo-half slots | hi-half slots]
        gt = np.zeros(2 * 128 * hc, dtype=np.int64)
        gt[:nlo] = dlo_
        gt[128 * hc : 128 * hc + nhi] = dhi_
        idx_t[b] = gt
        # dst-local slot values for S build (pad = 300 -> zero one-hot col)
        dl = np.full(2 * 128 * hc, 300, dtype=np.int64)
        dl[:nlo] = dlo_ % 128
        dl[128 * hc : 128 * hc + nhi] = dhi_ % 128
        dstloc[:, b * 2 * hc : (b + 1) * 2 * hc] = (
            dl.reshape(2 * hc, 128).T.astype(np.float32)
        )

    gnode = graph_ids[lo_k : lo_k + NODES_PER_CORE].astype(np.float32)
    return {
        "idx_lo": np.concatenate([_wrap16(idx_lo[b]) for b in range(BLOCKS)], 1),
        "idx_hi": np.concatenate([_wrap16(idx_hi[b]) for b in range(BLOCKS)], 1),
        "idx_t": np.concatenate([_wrap16(idx_t[b]) for b in range(BLOCKS)], 1),
        "dstloc": dstloc.astype(BF16),
        "gnode": gnode.reshape(BLOCKS, 128).T.astype(np.float32).copy(),  # [128, BLOCKS]
    }


def _build(hc):
    import concourse.bass as bass
    import concourse.bacc as bacc
    import concourse.mybir as mybir
    import concourse.tile as tile
    from concourse.masks import make_identity

    fp32 = mybir.dt.float32
    bf16 = mybir.dt.bfloat16
    i16 = mybir.dt.int16
    AF = mybir.ActivationFunctionType

    C2 = 2 * hc  # cols per block
    W16 = 128 * hc // 16  # idx cols per half-gather
    TW16 = 2 * W16

    nc = bacc.Bacc(
        "TRN2",
        target_bir_lowering=False,
        debug=False,
        num_devices=N_CORES,
        num_swdge_queues=4,
    )

    # ---- inputs ----
    xb = nc.dram_tensor("xb", [N, F_IN], bf16, kind="ExternalInput")
    xown = nc.dram_tensor("xown", [NODES_PER_CORE, F_IN], bf16, kind="ExternalInput")
    r0 = nc.dram_tensor("r0", [F_IN, 272], bf16, kind="ExternalInput")
    r1 = nc.dram_tensor("r1", [F_MID, 272], bf16, kind="ExternalInput")
    c0r = nc.dram_tensor("c0r", [F_IN, 8], bf16, kind="ExternalInput")
    wd1 = nc.dram_tensor("wd1", [F_MID, P_HID], bf16, kind="ExternalInput")
    bd1 = nc.dram_tensor("bd1", [P_HID, 1], fp32, kind="ExternalInput")
    wd2 = nc.dram_tensor("wd2", [P_HID, 1], bf16, kind="ExternalInput")
    bd2 = nc.dram_tensor("bd2", [1, 1], fp32, kind="ExternalInput")
    cntr = nc.dram_tensor("cntr", [128, G], fp32, kind="ExternalInput")
    iota128 = nc.dram_tensor("iota128", [128, 128], bf16, kind="ExternalInput")
    iota512 = nc.dram_tensor("iota512", [128, G], fp32, kind="ExternalInput")
    d_idx_lo = nc.dram_tensor("idx_lo", [128, BLOCKS * W16], i16, kind="ExternalInput")
    d_idx_hi = nc.dram_tensor("idx_hi", [128, BLOCKS * W16], i16, kind="ExternalInput")
    d_idx_t = nc.dram_tensor("idx_t", [128, BLOCKS * TW16], i16, kind="ExternalInput")
    d_dstloc = nc.dram_tensor("dstloc", [128, BLOCKS * C2], bf16, kind="ExternalInput")
    d_gnode = nc.dram_tensor("gnode", [128, BLOCKS], fp32, kind="ExternalInput")

    out_d = nc.dram_tensor("out", [G, 1], fp32, kind="ExternalOutput")
    import os
    _dbg = bool(os.environ.get("GAT_DEBUG"))
    if _dbg:
        dbg_tab0 = nc.dram_tensor("dbg_tab0", [512, ROW_W], bf16, kind="ExternalOutput")
        dbg_t0 = nc.dram_tensor("dbg_t0", [512, T_W], bf16, kind="ExternalOutput")
        dbg_slab = nc.dram_tensor("dbg_slab", [NODES_PER_CORE, ROW_W], bf16, kind="ExternalOutput")
        dbg_pool = nc.dram_tensor("dbg_pool", [F_MID, G], fp32, kind="ExternalOutput")

    # ---- internal DRAM ----
    t0tab = nc.dram_tensor("t0tab", [NODES_PER_CORE, T_W], bf16, kind="Internal")
    t1tab = nc.dram_tensor("t1tab", [NODES_PER_CORE, T_W], bf16, kind="Internal")
    tab0 = nc.dram_tensor("tab0", [N, ROW_W], bf16, kind="Internal")

    with tile.TileContext(nc) as tc:
        with (
            tc.tile_pool(name="const", bufs=1) as cp,
            tc.tile_pool(name="idxp", bufs=1) as ip,
            tc.tile_pool(name="xt", bufs=4) as xtp,
            tc.tile_pool(name="stage", bufs=4) as stp,
            tc.tile_pool(name="gath", bufs=4) as gp,
            tc.tile_pool(name="work", bufs=3) as wp,
            tc.tile_pool(name="mlp", bufs=1) as mp,
            tc.tile_pool(name="ps", bufs=1, space="PSUM") as pp,
            tc.tile_pool(name="psacc", bufs=1, space="PSUM") as pacc,
            tc.tile_pool(name="dram", bufs=1, space="DRAM") as dp,
        ):
            # ---- resident constants ----
            r0t = cp.tile([F_IN, 272], bf16)
            nc.sync.dma_start(out=r0t[:], in_=r0[:])
            r1t = cp.tile([128, 2, 272], bf16)
            nc.sync.dma_start(
                out=r1t[:], in_=r1[:].rearrange("(h p) w -> p h w", p=128)
            )
            c0rt = cp.tile([F_IN, 8], bf16)
            nc.sync.dma_start(out=c0rt[:], in_=c0r[:])
            io128 = cp.tile([128, 128], bf16)
            nc.sync.dma_start(out=io128[:], in_=iota128[:])
            io512 = cp.tile([128, G], fp32)
            nc.sync.dma_start(out=io512[:], in_=iota512[:])
            ident = cp.tile([128, 128], bf16)
            make_identity(nc, ident[:])
            ilo = ip.tile([128, BLOCKS * W16], i16)
            nc.sync.dma_start(out=ilo[:], in_=d_idx_lo[:])
            ihi = ip.tile([128, BLOCKS * W16], i16)
            nc.sync.dma_start(out=ihi[:], in_=d_idx_hi[:])
            it = ip.tile([128, BLOCKS * TW16], i16)
            nc.sync.dma_start(out=it[:], in_=d_idx_t[:])
            dlo = ip.tile([128, BLOCKS * C2], bf16)
            nc.sync.dma_start(out=dlo[:], in_=d_dstloc[:])
            gnd = ip.tile([128, BLOCKS], fp32)
            nc.sync.dma_start(out=gnd[:], in_=d_gnode[:])

            # ---- stage A: T0 build (all 65536 rows, redundant per core) ----
            # X^T tiles of 512 nodes; per tile 4 matmuls vs r0 -> [h|el|er]
            xview = xb[:].rearrange("(a n) f -> a n f", n=512)
            t0view = tab0[:].rearrange("(a g p) w -> a g p w", g=4, p=128)
            for a in range(N // 512):
                xt = xtp.tile([F_IN, 512], bf16, tag="xt")
                nc.sync.dma_start(out=xt[:], in_=xview[a], transpose=True)
                st = stp.tile([128, 4, ROW_W], bf16, tag="t0st")
                for g in range(4):
                    ps = pp.tile([128, 272], fp32, tag="mmbig", bufs=3)
                    nc.tensor.matmul(
                        out=ps[:],
                        lhsT=xt[:, g * 128 : (g + 1) * 128],
                        rhs=r0t[:],
                        start=True,
                        stop=True,
                    )
                    nc.scalar.copy(out=st[:, g, 0:256], in_=ps[:, 0:256])
                    nc.scalar.activation(
                        out=st[:, g, 256:264], in_=ps[:, 256:264], func=AF.Exp
                    )
                    nc.scalar.activation(
                        out=st[:, g, 264:272], in_=ps[:, 256:264], func=AF.Exp,
                        scale=0.2,
                    )
                nc.sync.dma_start(out=t0view[a].rearrange("g p w -> p g w"), in_=st[:])

            # t0 table for own dst range, from xown
            xoview = xown[:].rearrange("(a n) f -> a n f", n=512)
            t0tview = t0tab[:].rearrange("(a g p) w -> a g p w", g=4, p=128)
            for a in range(NODES_PER_CORE // 512):
                xt = xtp.tile([F_IN, 512], bf16, tag="xt")
                nc.sync.dma_start(out=xt[:], in_=xoview[a], transpose=True)
                st = stp.tile([128, 4, T_W], bf16, tag="tst")
                for g in range(4):
                    ps = pp.tile([128, 8], fp32, tag="small", bufs=2)
                    nc.tensor.matmul(
                        out=ps[:],
                        lhsT=xt[:, g * 128 : (g + 1) * 128],
                        rhs=c0rt[:],
                        start=True,
                        stop=True,
                    )
                    nc.scalar.activation(
                        out=st[:, g, 0:8], in_=ps[:], func=AF.Exp, scale=-0.8
                    )
                nc.sync.dma_start(out=t0tview[a].rearrange("g p w -> p g w"), in_=st[:])

            # ---- layer loops ----
            t1slab = dp.tile([NODES_PER_CORE, ROW_W], bf16)
            t1full = dp.tile([N, ROW_W], bf16)
            poolin = dp.tile([F_MID, G], fp32)
            poolout = dp.tile([F_MID, G], fp32)

            def gat_layer(tab_lo, tab_hi, ttab, layer):
                """One GAT layer over 64 dst blocks."""
                pooled = None
                if layer == 1:
                    pool_a = pacc.tile([128, G], fp32, tag="pool_a")
                    pool_b = pacc.tile([128, G], fp32, tag="pool_b")
                    pooled = [pool_a, pool_b]
                t1view = t1slab[:].rearrange("(b p) w -> b p w", p=128)
                t1tview = t1tab[:].rearrange("(b p) w -> b p w", p=128)
                for b in range(BLOCKS):
                    g = gp.tile([128, C2, ROW_W], bf16, tag="g")
                    nc.gpsimd.dma_gather(
                        out_ap=g[:, :hc, :],
                        in_ap=tab_lo,
                        idxs_ap=ilo[:, b * W16 : (b + 1) * W16],
                        num_idxs=128 * hc,
                        num_idxs_reg=128 * hc,
                        elem_size=ROW_W,
                        single_packet=False,
                        queue_num=(3 * b) % 4,
                    )
                    nc.gpsimd.dma_gather(
                        out_ap=g[:, hc:, :],
                        in_ap=tab_hi,
                        idxs_ap=ihi[:, b * W16 : (b + 1) * W16],
                        num_idxs=128 * hc,
                        num_idxs_reg=128 * hc,
                        elem_size=ROW_W,
                        single_packet=False,
                        queue_num=(3 * b + 1) % 4,
                    )
                    tb = gp.tile([128, C2, T_W], bf16, tag="tb")
                    nc.gpsimd.dma_gather(
                        out_ap=tb[:],
                        in_ap=ttab,
                        idxs_ap=it[:, b * TW16 : (b + 1) * TW16],
                        num_idxs=128 * C2,
                        num_idxs_reg=128 * C2,
                        elem_size=T_W,
                        single_packet=False,
                        queue_num=(3 * b + 2) % 4,
                    )
                    # m~ = max(p, u*t) -> rhs[:, :, 0:8]
                    rhs = wp.tile([128, C2, 264], bf16, tag="rhs")
                    ut = wp.tile([128, C2, 8], fp32, tag="ut")
                    nc.vector.tensor_tensor(
                        out=ut[:],
                        in0=g[:, :, 264:272],
                        in1=tb[:, :, 0:8],
                        op=mybir.AluOpType.mult,
                    )
                    nc.vector.tensor_tensor(
                        out=rhs[:, :, 0:8],
                        in0=ut[:],
                        in1=g[:, :, 256:264],
                        op=mybir.AluOpType.max,
                    )
                    # msg = m~ (bcast over 32) * h -> rhs[:, :, 8:264]
                    mb_ = rhs[:, :, 0:8][:, :, :, None].to_broadcast([128, C2, 8, 32])
                    nc.vector.tensor_tensor(
                        out=rhs[:, :, 8:264].rearrange("p c (h d) -> p c h d", d=32),
                        in0=g[:, :, 0:256].rearrange("p c (h d) -> p c h d", d=32),
                        in1=mb_,
                        op=mybir.AluOpType.mult,
                    )
                    # S one-hot [128, C2, 128]
                    s = wp.tile([128, C2, 128], bf16, tag="s")
                    nc.vector.tensor_tensor(
                        out=s[:],
                        in0=dlo[:, b * C2 : (b + 1) * C2][:, :, None]
                        .to_broadcast([128, C2, 128]),
                        in1=io128[:][:, None, :].to_broadcast([128, C2, 128]),
                        op=mybir.AluOpType.is_equal,
                    )
                    acc = pp.tile([128, 264], fp32, tag="mmbig", bufs=3)
                    for c in range(C2):
                        nc.tensor.matmul(
                            out=acc[:],
                            lhsT=s[:, c, :],
                            rhs=rhs[:, c, :],
                            start=(c == 0),
                            stop=(c == C2 - 1),
                        )
                    # normalize: h_out = acc[:, 8:264] / denom
                    rec = wp.tile([128, 8], fp32, tag="rec")
                    nc.vector.tensor_scalar_max(out=rec[:], in0=acc[:, 0:8], scalar1=1e-30)
                    nc.vector.reciprocal(out=rec[:], in_=rec[:])
                    hout = wp.tile([128, F_MID], bf16, tag="hout")
                    nc.vector.tensor_tensor(
                        out=hout[:].rearrange("p (h d) -> p h d", d=32),
                        in0=acc[:, 8:264].rearrange("p (h d) -> p h d", d=32),
                        in1=rec[:][:, :, None].to_broadcast([128, 8, 32]),
                        op=mybir.AluOpType.mult,
                    )
                    if layer == 0:
                        # h^T via PE transpose, then [h1|el1|er1] = h^T.T @ r1
                        hT = wp.tile([128, 2, 128], bf16, tag="hT")
                        for fh in range(2):
                            pst = pp.tile([128, 128], bf16, tag="small", bufs=2)
                            nc.tensor.transpose(
                                out=pst[:],
                                in_=hout[:, fh * 128 : (fh + 1) * 128],
                                identity=ident[:],
                            )
                            nc.vector.tensor_copy(out=hT[:, fh, :], in_=pst[:])
                        ps2 = pp.tile([128, 272], fp32, tag="mmbig", bufs=3)
                        for fh in range(2):
                            nc.tensor.matmul(
                                out=ps2[:],
                                lhsT=hT[:, fh, :],
                                rhs=r1t[:, fh, :],
                                start=(fh == 0),
                                stop=(fh == 1),
                            )
                        st = stp.tile([128, 272], bf16, tag="t1st")
                        nc.vector.tensor_copy(out=st[:, 0:256], in_=ps2[:, 0:256])
                        nc.scalar.activation(
                            out=st[:, 256:264], in_=ps2[:, 256:264], func=AF.Exp
                        )
                        nc.scalar.activation(
                            out=st[:, 264:272],
                            in_=ps2[:, 256:264],
                            func=AF.Exp,
                            scale=0.2,
                        )
                        tst = stp.tile([128, 8], bf16, tag="tt1")
                        nc.scalar.activation(
                            out=tst[:], in_=ps2[:, 264:272], func=AF.Exp, scale=-0.8
                        )
                        nc.sync.dma_start(out=t1view[b, :, 0:272], in_=st[:])
                        nc.sync.dma_start(out=t1tview[b, :, 0:8], in_=tst[:])
                    else:
                        # pooling: pooledT[f, g] += hout^T-slices @ P
                        pmat = wp.tile([128, G], bf16, tag="pmat")
                        nc.vector.tensor_tensor(
                            out=pmat[:],
                            in0=gnd[:, b : b + 1].to_broadcast([128, G]),
                            in1=io512[:],
                            op=mybir.AluOpType.is_equal,
                        )
                        for fh in range(2):
                            nc.tensor.matmul(
                                out=pooled[fh][:],
                                lhsT=hout[:, fh * 128 : (fh + 1) * 128],
                                rhs=pmat[:],
                                start=(b == 0),
                                stop=(b == BLOCKS - 1),
                            )
                return pooled

            gat_layer(tab0[:HALF, :], tab0[HALF:, :], t0tab[:], 0)

            # AllGather slab -> full table
            nc.gpsimd.collective_compute(
                "AllGather",
                mybir.AluOpType.bypass,
                ins=[t1slab.opt()],
                outs=[t1full.opt()],
                replica_groups=[list(range(N_CORES))],
            )

            pooled = gat_layer(
                t1full[:][:HALF, :], t1full[:][HALF:, :], t1tab[:], 1
            )

            if _dbg:
                nc.sync.dma_start(out=dbg_tab0[:], in_=tab0[0:512, :])
                nc.sync.dma_start(out=dbg_t0[:], in_=t0tab[0:512, :])
                nc.sync.dma_start(out=dbg_slab[:], in_=t1slab[:])

            # ---- pooled AllReduce ----
            pst = mp.tile([128, 2, G], fp32)
            nc.vector.tensor_copy(out=pst[:, 0, :], in_=pooled[0][:])
            nc.vector.tensor_copy(out=pst[:, 1, :], in_=pooled[1][:])
            nc.sync.dma_start(
                out=poolin[:].rearrange("(h p) g -> p h g", p=128), in_=pst[:]
            )
            nc.gpsimd.collective_compute(
                "AllReduce",
                mybir.AluOpType.add,
                ins=[poolin.opt()],
                outs=[poolout.opt()],
                replica_groups=[list(range(N_CORES))],
            )
            if _dbg:
                nc.sync.dma_start(out=dbg_pool[:], in_=poolin[:])

            # ---- MLP ----
            cnt = mp.tile([128, G], fp32)
            nc.sync.dma_start(out=cnt[:], in_=cntr[:])
            mean = mp.tile([128, 2, G], bf16)
            pr = mp.tile([128, 2, G], fp32)
            nc.sync.dma_start(
                out=pr[:], in_=poolout[:].rearrange("(h p) g -> p h g", p=128)
            )
            for fh in range(2):
                nc.vector.tensor_tensor(
                    out=mean[:, fh, :],
                    in0=pr[:, fh, :],
                    in1=cnt[:],
                    op=mybir.AluOpType.mult,
                )
            w1t = mp.tile([128, 2, P_HID], bf16)
            nc.sync.dma_start(
                out=w1t[:], in_=wd1[:].rearrange("(h p) j -> p h j", p=128)
            )
            b1t = mp.tile([128, 4, 1], fp32)
            nc.sync.dma_start(
                out=b1t[:], in_=bd1[:].rearrange("(q p) o -> p q o", p=128)
            )
            w2t = mp.tile([128, 4, 1], bf16)
            nc.sync.dma_start(
                out=w2t[:], in_=wd2[:].rearrange("(q p) o -> p q o", p=128)
            )
            b2t = mp.tile([1, 1], fp32)
            nc.sync.dma_start(out=b2t[:], in_=bd2[:])
            hid = mp.tile([128, 4, G], bf16)
            for q in range(4):
                psh = pp.tile([128, G], fp32, tag="mmbig", bufs=3)
                for fh in range(2):
                    nc.tensor.matmul(
                        out=psh[:],
                        lhsT=w1t[:, fh, q * 128 : (q + 1) * 128],
                        rhs=mean[:, fh, :],
                        start=(fh == 0),
                        stop=(fh == 1),
                    )
                nc.scalar.activation(
                    out=hid[:, q, :],
                    in_=psh[:],
                    func=AF.Relu,
                    bias=b1t[:, q, :],
                )
            pso = pp.tile([1, G], fp32, tag="psout", bufs=1)
            for q in range(4):
                nc.tensor.matmul(
                    out=pso[:],
                    lhsT=w2t[:, q, :],
                    rhs=hid[:, q, :],
                    start=(q == 0),
                    stop=(q == 3),
                )
            ot = mp.tile([1, G], fp32)
            nc.vector.tensor_scalar_add(out=ot[:], in0=pso[:], scalar1=b2t[:])
            nc.sync.dma_start(out=out_d[:].rearrange("g o -> o g"), in_=ot[:])

    nc.compile()
    return nc


_CACHED = {}


def kernel(node_feats, src, dst, graph_ids, num_graphs,
           W0, al0, ar0, W1, al1, ar1, Wd1, bd1, Wd2, bd2,
           _trace=False):
    from concourse.bass_utils import run_bass_kernel_spmd

    src = np.asarray(src).astype(np.int64)
    dst = np.asarray(dst).astype(np.int64)
    graph_ids = np.asarray(graph_ids).astype(np.int64)
    node_feats = np.asarray(node_feats, dtype=np.float32)

    # ---- host prep ----
    # attention-projection matrices folded into the fc weights
    def _amat(al):
        a = np.zeros((F_MID, H), np.float32)
        for h_ in range(H):
            a[h_ * D : (h_ + 1) * D, h_] = al[h_]
        return a

    W0 = np.asarray(W0, np.float32)
    W1 = np.asarray(W1, np.float32)
    r0 = np.concatenate(
        [W0, W0 @ _amat(np.asarray(al0, np.float32)),
         W0 @ _amat(np.asarray(ar0, np.float32))], axis=1
    ).astype(BF16)
    r1 = np.concatenate(
        [W1, W1 @ _amat(np.asarray(al1, np.float32)),
         W1 @ _amat(np.asarray(ar1, np.float32))], axis=1
    ).astype(BF16)
    c0r = (W0 @ _amat(np.asarray(ar0, np.float32))).astype(BF16)

    cnt = np.bincount(graph_ids, minlength=G).astype(np.float32)
    cntr = np.tile((1.0 / np.maximum(cnt, 1.0))[None, :], (128, 1)).astype(np.float32)
    iota128 = np.tile(np.arange(128, dtype=np.float32)[None, :], (128, 1)).astype(BF16)
    iota512 = np.tile(np.arange(G, dtype=np.float32)[None, :], (128, 1))
    xb = node_feats.astype(BF16)

    # per-(core, block, half) max edge count -> hc
    counts = np.zeros((N_CORES, BLOCKS, 2), np.int64)
    core = dst // NODES_PER_CORE
    blk = (dst % NODES_PER_CORE) // 128
    half = (src >= HALF).astype(np.int64)
    np.add.at(counts, (core, blk, half), 1)
    hc = int(np.ceil(counts.max() / 128))

    key = hc
    if key not in _CACHED:
        _CACHED[key] = _build(hc)
    nc = _CACHED[key]

    shared = {
        "xb": xb, "r0": r0, "r1": r1, "c0r": c0r,
        "wd1": np.asarray(Wd1, np.float32).astype(BF16),
        "bd1": np.asarray(bd1, np.float32).reshape(P_HID, 1),
        "wd2": np.asarray(Wd2, np.float32).astype(BF16).reshape(P_HID, 1),
        "bd2": np.asarray(bd2, np.float32).reshape(1, 1),
        "cntr": cntr, "iota128": iota128, "iota512": iota512,
    }
    in_maps = []
    for k in range(N_CORES):
        m = dict(shared)
        m["xown"] = xb[k * NODES_PER_CORE : (k + 1) * NODES_PER_CORE]
        m.update(_prep_core(k, src, dst, graph_ids, hc))
        in_maps.append(m)

    res = run_bass_kernel_spmd(
        nc, in_maps, core_ids=list(range(N_CORES)), trace=_trace
    )
    out = res.results[0]["out"].astype(np.float32)
    if _trace:
        kernel._last_exec_ns = res.exec_time_ns
    return out
